# revision 1
# baseline (speedup 1.0000x reference)
"""Trainium2 Bass kernel for DifferentiableKMeans retrieval-knn.

Computes, for x [32768, 128] and cluster_centers [512, 128]:
    d2 = ||x||^2 - 2 x@c.T + ||c||^2          [N, 512]
    idx = top-10 smallest-distance cluster ids  [N, 10]
    out = x[idx].reshape(1, N*10, 128)          (gather of x rows 0..511)

Sharding: data-parallel over N across 8 NeuronCores; centers and the
gather table (x[:512]) replicated.

Per-core pipeline (4096 points, 32 tiles of 128):
  PE:  transpose(x_tile) -> fp32 matmul m = x@cT, + K=1 matmul adds -0.5*|c|^2
  ACT: PSUM->SBUF copies
  DVE: max8 / max_index / match_replace / max8 / max_index  => top-10 indices
  Pool/DMA: indirect DMA gathers x rows (512B each) into SBUF, then one
  contiguous 655KB store per tile into the final output layout.

Ranking is by m = x.c - 0.5*|c|^2 (monotone equivalent of distance per row).
"""

import os
import sys

for _p in ("/opt/trn_rl_repo", "/root/.axon_site/_ro/trn_rl_repo"):
    if os.path.isdir(_p) and _p not in sys.path:
        sys.path.insert(0, _p)

import numpy as np

N_FULL = 32768
D = 128
K = 512
TOPK = 10
N_CORES = 8
N_SHARD = N_FULL // N_CORES  # 4096
P = 128

_BUILD_CACHE = {}


def build_nc(n_points=N_SHARD):
    """Build (and compile) the single-core Bass module for a shard of
    `n_points` points. Returns the compiled Bacc instance."""
    if n_points in _BUILD_CACHE:
        return _BUILD_CACHE[n_points]

    import concourse.bass as bass
    import concourse.mybir as mybir
    from concourse import bacc
    from concourse.masks import make_identity
    from concourse.tile import TileContext

    f32 = mybir.dt.float32
    AFT = mybir.ActivationFunctionType
    nt = n_points // P
    assert n_points % P == 0

    nc = bacc.Bacc("TRN2", target_bir_lowering=False, debug=False)

    x = nc.dram_tensor("x", [n_points, D], f32, kind="ExternalInput")
    xhead = nc.dram_tensor("xhead", [K, D], f32, kind="ExternalInput")
    cc = nc.dram_tensor("cluster_centers", [K, D], f32, kind="ExternalInput")
    out = nc.dram_tensor("out", [n_points * TOPK, D], f32, kind="ExternalOutput")

    with TileContext(nc) as tc:
        with tc.tile_pool(name="const", bufs=1) as const_pool:
            identity = const_pool.tile([P, P], f32)
            make_identity(nc, identity[:])

            cT = const_pool.tile([P, K], f32)          # centers transposed [d, k]
            negc2 = const_pool.tile([1, K], f32)       # -0.5*|c_k|^2 row
            ones_row = const_pool.tile([1, P], f32)
            nc.vector.memset(ones_row[:], 1.0)
            c2col = const_pool.tile([P, K // P], f32)

            # REP[k, m] = 1 iff m % 16 == k: replicates a [16, S] tile to all
            # 8 Q7-core partition blocks via matmul (out[m] = in[m % 16]).
            rep = const_pool.tile([16, P], f32)
            rep3 = rep[:].rearrange("k (u l) -> k u l", l=16)
            nc.gpsimd.memset(rep3, 0.0)
            # iota val = k - l; keep 0.0 where val != 0, write 1.0 where k == l
            nc.gpsimd.affine_select(
                out=rep3, in_=rep3,
                compare_op=mybir.AluOpType.not_equal, fill=1.0,
                base=0, pattern=[[0, P // 16], [-1, 16]], channel_multiplier=1,
            )

            with tc.tile_pool(name="prep", bufs=2) as prep_pool, \
                 tc.tile_pool(name="prep_ps", bufs=2, space="PSUM") as prep_psum:
                c2row_ps = prep_psum.tile([1, K], f32, tag="c2row")
                for j in range(K // P):
                    cchunk = prep_pool.tile([P, P], f32, tag="cchunk")
                    nc.sync.dma_start(cchunk[:], cc[j * P:(j + 1) * P, :])
                    pst = prep_psum.tile([P, P], f32, tag="pst")
                    nc.tensor.transpose(pst[:], cchunk[:], identity[:])
                    nc.scalar.copy(cT[:, j * P:(j + 1) * P], pst[:])
                    sq = prep_pool.tile([P, P], f32, tag="sq")
                    nc.scalar.activation(
                        sq[:], cchunk[:], AFT.Square,
                        accum_out=c2col[:, j:j + 1],
                    )
                    # transpose the [P,1] column of |c|^2 into a [1,P] row chunk
                    nc.tensor.matmul(
                        c2row_ps[0:1, j * P:(j + 1) * P],
                        lhsT=c2col[:, j:j + 1], rhs=identity[:],
                        start=True, stop=True,
                    )
                nc.scalar.activation(negc2[:], c2row_ps[:], AFT.Copy, scale=-0.5)

            with tc.tile_pool(name="xin", bufs=3) as xin_pool, \
                 tc.tile_pool(name="xt", bufs=3) as xt_pool, \
                 tc.tile_pool(name="ms", bufs=3) as ms_pool, \
                 tc.tile_pool(name="ms2", bufs=3) as ms2_pool, \
                 tc.tile_pool(name="small", bufs=4) as small_pool, \
                 tc.tile_pool(name="gath", bufs=4) as gath_pool, \
                 tc.tile_pool(name="mm_ps", bufs=2, space="PSUM") as mm_psum, \
                 tc.tile_pool(name="wrap_ps", bufs=1, space="PSUM") as wrap_psum, \
                 tc.tile_pool(name="tr_ps", bufs=2, space="PSUM") as tr_psum:
                for i in range(nt):
                    x_tile = xin_pool.tile([P, D], f32, tag="x")
                    nc.sync.dma_start(x_tile[:], x[i * P:(i + 1) * P, :])

                    pst = tr_psum.tile([P, P], f32, tag="xtp")
                    nc.tensor.transpose(pst[:], x_tile[:], identity[:])
                    xT = xt_pool.tile([P, P], f32, tag="xT")
                    nc.scalar.copy(xT[:], pst[:])

                    pm = mm_psum.tile([P, K], f32, tag="pm")
                    nc.tensor.matmul(pm[:], lhsT=xT[:], rhs=cT[:],
                                     start=True, stop=False)
                    nc.tensor.matmul(pm[:], lhsT=ones_row[:], rhs=negc2[:],
                                     start=False, stop=True)

                    ms = ms_pool.tile([P, K], f32, tag="ms")
                    nc.scalar.copy(ms[:], pm[:])

                    v8 = small_pool.tile([P, 8], f32, tag="v8")
                    nc.vector.max(v8[:], ms[:])
                    idx = small_pool.tile([P, 16], mybir.dt.uint32, tag="idx")
                    nc.vector.max_index(idx[:, 0:8], v8[:], ms[:])
                    ms2 = ms2_pool.tile([P, K], f32, tag="ms2")
                    nc.vector.match_replace(
                        out=ms2[:], in_to_replace=v8[:], in_values=ms[:],
                        imm_value=-1e30,
                    )
                    v8b = small_pool.tile([P, 8], f32, tag="v8b")
                    nc.vector.max(v8b[:], ms2[:])
                    nc.vector.max_index(idx[:, 8:16], v8b[:], ms2[:])

                    # ---- build the 16-partition-wrapped int16 index list ----
                    # dma_gather consumes L[j] = W[j % 16, j // 16] and lands
                    # row j at out[j % 128, j // 128].  With j = r*128 + p the
                    # landing spot is (p, r), so we need
                    #   W[l, r*8 + u] = idx[l + 16*u, r]      (p = l + 16u)
                    # replicated across the 8 Q7-core partition blocks.
                    idxf = small_pool.tile([P, TOPK], f32, tag="idxf")
                    nc.scalar.copy(idxf[:], idx[:, 0:TOPK])  # uint32 -> f32
                    t_ps = wrap_psum.tile([TOPK, P], f32, tag="t_ps")
                    nc.tensor.transpose(t_ps[:], idxf[:], identity[:])
                    t_sb = small_pool.tile([TOPK, P], f32, tag="t_sb")
                    nc.scalar.copy(t_sb[:], t_ps[:])
                    w16_ps = wrap_psum.tile([16, 8 * TOPK], f32, tag="w16")
                    for u in range(8):
                        # [10, 16] slice -> transposed into columns r*8 + u
                        nc.tensor.transpose(
                            w16_ps[:, u:8 * TOPK:8],
                            t_sb[:, 16 * u:16 * (u + 1)],
                            identity[0:TOPK, 0:TOPK],
                        )
                    w16_sb = small_pool.tile([16, 8 * TOPK], f32, tag="w16sb")
                    nc.scalar.copy(w16_sb[:], w16_ps[:])
                    wrep_ps = wrap_psum.tile([P, 8 * TOPK], f32, tag="wrep")
                    nc.tensor.matmul(wrep_ps[:], lhsT=rep[:], rhs=w16_sb[:],
                                     start=True, stop=True)
                    widx = small_pool.tile([P, 8 * TOPK], mybir.dt.int16,
                                           tag="widx")
                    nc.scalar.copy(widx[:], wrep_ps[:])  # f32 -> int16

                    g = gath_pool.tile([P, TOPK * D], f32, tag="g")
                    nc.gpsimd.dma_gather(
                        out_ap=g[:].rearrange("p (r d) -> p r d", r=TOPK),
                        in_ap=xhead[:],
                        idxs_ap=widx[:],
                        num_idxs=P * TOPK,
                        num_idxs_reg=P * TOPK,
                        elem_size=D,
                        single_packet=False,
                    )

                    out_view = out[i * P * TOPK:(i + 1) * P * TOPK, :].rearrange(
                        "(p r) d -> p (r d)", p=P)
                    nc.sync.dma_start(out_view, g[:])

    nc.compile()
    _BUILD_CACHE[n_points] = nc
    return nc


def run_on_cores(x_np, cc_np, trace=False):
    """Run the SPMD kernel on all 8 cores. Returns (out [N,10,D], results)."""
    from concourse import bass_utils

    nc = build_nc(N_SHARD)
    xhead = np.ascontiguousarray(x_np[:K])
    in_maps = [
        {
            "x": np.ascontiguousarray(x_np[c * N_SHARD:(c + 1) * N_SHARD]),
            "xhead": xhead,
            "cluster_centers": cc_np,
        }
        for c in range(N_CORES)
    ]
    res = bass_utils.run_bass_kernel_spmd(
        nc, in_maps, core_ids=list(range(N_CORES)), trace=trace,
    )
    shards = [res.results[c]["out"] for c in range(N_CORES)]
    full = np.concatenate(shards, axis=0)  # [N*10, D]
    return full, res


def kernel(x, cluster_centers):
    x_np = np.ascontiguousarray(np.asarray(x, dtype=np.float32))
    cc_np = np.ascontiguousarray(np.asarray(cluster_centers, dtype=np.float32))
    full, _ = run_on_cores(x_np, cc_np, trace=False)
    return full.reshape(1, N_FULL * TOPK, D)



# revision 8
# speedup vs baseline: 1.0726x; 1.0726x over previous
"""Trainium2 Bass kernel for DifferentiableKMeans retrieval-knn.

Computes, for x [32768, 128] and cluster_centers [512, 128]:
    d2 = ||x||^2 - 2 x@c.T + ||c||^2          [N, 512]
    idx = top-10 smallest-distance cluster ids  [N, 10]
    out = x[idx].reshape(1, N*10, 128)          (gather of x rows 0..511)

Sharding: data-parallel over N across 8 NeuronCores; centers and the
gather table (x[:512], pre-rounded to bf16 on host) replicated.

Per-core pipeline (4096 points, 32 tiles of 128):
  PE:  fp32 matmul m = x@cT, + K=1 matmul adds -0.5*|c|^2 (fp32r was
       tried and is a reduced-precision mode: rel err 0.11, rejected).
  DVE: max8 / max_index / match_replace / max8 / max_index => top-10
       indices (reads scores straight from PSUM).
  ACT: small copies, bf16 -> fp32 upcast of the gathered rows.
  GpSimd: 2 x 640-index dma_gather per tile (bf16 rows, 256B each),
       rotated over 4 SWDGE queues with deep buffering -- measured ~3x
       faster than one 1280-index gather on a single queue.

Ranking is by m = x.c - 0.5*|c|^2 (monotone equivalent of distance per
row). Gathered values are bf16-rounded (rel err ~1e-3, well under the
2e-2 gate); the output tensor itself is fp32.
"""

import os
import sys

for _p in ("/opt/trn_rl_repo", "/root/.axon_site/_ro/trn_rl_repo"):
    if os.path.isdir(_p) and _p not in sys.path:
        sys.path.insert(0, _p)

import numpy as np

N_FULL = 32768
D = 128
K = 512
TOPK = 10
N_CORES = 8
N_SHARD = N_FULL // N_CORES  # 4096
P = 128

_BUILD_CACHE = {}


def build_nc(n_points=N_SHARD):
    """Build (and compile) the single-core Bass module for a shard of
    `n_points` points. Returns the compiled Bacc instance."""
    if n_points in _BUILD_CACHE:
        return _BUILD_CACHE[n_points]

    import concourse.bass as bass
    import concourse.mybir as mybir
    from concourse import bacc
    from concourse.masks import make_identity
    from concourse.tile import TileContext

    f32 = mybir.dt.float32
    f32r = mybir.dt.float32r
    f16 = mybir.dt.float16
    bf16 = mybir.dt.bfloat16
    AFT = mybir.ActivationFunctionType
    nt = n_points // P
    assert n_points % P == 0

    nc = bacc.Bacc("TRN2", target_bir_lowering=False, debug=False,
                   num_swdge_queues=4)

    x = nc.dram_tensor("x", [n_points, D], f32, kind="ExternalInput")
    xheadb = nc.dram_tensor("xheadb", [K, D], bf16, kind="ExternalInput")
    cc = nc.dram_tensor("cluster_centers", [K, D], f32, kind="ExternalInput")
    out = nc.dram_tensor("out", [n_points * TOPK, D], f32, kind="ExternalOutput")

    with TileContext(nc) as tc:
        with tc.tile_pool(name="const", bufs=1) as const_pool:
            identity = const_pool.tile([P, P], f32)
            make_identity(nc, identity[:])

            cT = const_pool.tile([P, K], f32)          # centers transposed [d, k]
            negc2 = const_pool.tile([1, K], f32)       # -0.5*|c_k|^2 row
            ones_row = const_pool.tile([1, P], f32)
            nc.vector.memset(ones_row[:], 1.0)
            c2col = const_pool.tile([P, K // P], f32)

            # REP[k, m] = 1 iff m % 16 == k: replicates a [16, S] tile to all
            # 8 Q7-core partition blocks via matmul (out[m] = in[m % 16]).
            rep = const_pool.tile([16, P], f32)
            rep3 = rep[:].rearrange("k (u l) -> k u l", l=16)
            nc.gpsimd.memset(rep3, 0.0)
            # iota val = k - l; keep 0.0 where val != 0, write 1.0 where k == l
            nc.gpsimd.affine_select(
                out=rep3, in_=rep3,
                compare_op=mybir.AluOpType.not_equal, fill=1.0,
                base=0, pattern=[[0, P // 16], [-1, 16]], channel_multiplier=1,
            )

            with tc.tile_pool(name="prep", bufs=2) as prep_pool, \
                 tc.tile_pool(name="prep_ps", bufs=2, space="PSUM") as prep_psum:
                c2row_ps = prep_psum.tile([1, K], f32, tag="c2row")
                for j in range(K // P):
                    cchunk = prep_pool.tile([P, P], f32, tag="cchunk")
                    nc.sync.dma_start(cchunk[:], cc[j * P:(j + 1) * P, :])
                    pst = prep_psum.tile([P, P], f32, tag="pst")
                    nc.tensor.transpose(pst[:], cchunk[:], identity[:])
                    nc.scalar.copy(cT[:, j * P:(j + 1) * P], pst[:])
                    sq = prep_pool.tile([P, P], f32, tag="sq")
                    nc.scalar.activation(
                        sq[:], cchunk[:], AFT.Square,
                        accum_out=c2col[:, j:j + 1],
                    )
                    # transpose the [P,1] column of |c|^2 into a [1,P] row chunk
                    nc.tensor.matmul(
                        c2row_ps[0:1, j * P:(j + 1) * P],
                        lhsT=c2col[:, j:j + 1], rhs=identity[:],
                        start=True, stop=True,
                    )
                nc.scalar.activation(negc2[:], c2row_ps[:], AFT.Copy, scale=-0.5)

            with tc.tile_pool(name="xin", bufs=3) as xin_pool, \
                 tc.tile_pool(name="xt", bufs=3) as xt_pool, \
                 tc.tile_pool(name="ms2", bufs=3) as ms2_pool, \
                 tc.tile_pool(name="small", bufs=4) as small_pool, \
                 tc.tile_pool(name="gath", bufs=8) as gath_pool, \
                 tc.tile_pool(name="gf", bufs=4) as gf_pool, \
                 tc.tile_pool(name="mm_ps", bufs=3, space="PSUM") as mm_psum, \
                 tc.tile_pool(name="wrap_ps", bufs=1, space="PSUM") as wrap_psum, \
                 tc.tile_pool(name="tr_ps", bufs=2, space="PSUM") as tr_psum:
                for i in range(nt):
                    x_tile = xin_pool.tile([P, D], f32, tag="x")
                    nc.sync.dma_start(x_tile[:], x[i * P:(i + 1) * P, :])

                    pst = tr_psum.tile([P, P], f32, tag="xtp")
                    nc.tensor.transpose(pst[:], x_tile[:], identity[:])
                    xT = xt_pool.tile([P, P], f32, tag="xT")
                    nc.scalar.copy(xT[:], pst[:])

                    pm = mm_psum.tile([P, K], f32, tag="pm")
                    nc.tensor.matmul(pm[:], lhsT=xT[:], rhs=cT[:],
                                     start=True, stop=False)
                    nc.tensor.matmul(pm[:], lhsT=ones_row[:], rhs=negc2[:],
                                     start=False, stop=True)

                    # top-10 per point, straight off PSUM
                    v8 = small_pool.tile([P, 8], f32, tag="v8")
                    nc.vector.max(v8[:], pm[:])
                    idx = small_pool.tile([P, 16], mybir.dt.uint32, tag="idx")
                    nc.vector.max_index(idx[:, 0:8], v8[:], pm[:])
                    ms2 = ms2_pool.tile([P, K], f32, tag="ms2")
                    nc.vector.match_replace(
                        out=ms2[:], in_to_replace=v8[:], in_values=pm[:],
                        imm_value=-1e30,
                    )
                    v8b = small_pool.tile([P, 8], f32, tag="v8b")
                    nc.vector.max(v8b[:], ms2[:])
                    nc.vector.max_index(idx[:, 8:16], v8b[:], ms2[:])

                    # ---- build the 16-partition-wrapped int16 index list ----
                    # dma_gather consumes L[j] = W[j % 16, j // 16] and lands
                    # row j at out[j % 128, j // 128].  With j = r*128 + p the
                    # landing spot is (p, r), so we need
                    #   W[l, r*8 + u] = idx[l + 16*u, r]      (p = l + 16u)
                    # replicated across the 8 Q7-core partition blocks.
                    # fp16 keeps index values (< 512) exact and transposes at
                    # half the fp32 PE cost.
                    idxf = small_pool.tile([P, TOPK], f32, tag="idxf")
                    nc.scalar.copy(idxf[:], idx[:, 0:TOPK])  # uint32 -> f16
                    t_ps = wrap_psum.tile([TOPK, P], f32, tag="t_ps")
                    nc.tensor.transpose(t_ps[:], idxf[:], identity[:])
                    t_sb = small_pool.tile([TOPK, P], f32, tag="t_sb")
                    nc.scalar.copy(t_sb[:], t_ps[:])
                    w16_ps = wrap_psum.tile([16, 8 * TOPK], f32, tag="w16")
                    for u in range(8):
                        # [10, 16] slice -> transposed into columns r*8 + u
                        nc.tensor.transpose(
                            w16_ps[:, u:8 * TOPK:8],
                            t_sb[:, 16 * u:16 * (u + 1)],
                            identity[0:TOPK, 0:TOPK],
                        )
                    w16_sb = small_pool.tile([16, 8 * TOPK], f32, tag="w16sb")
                    nc.scalar.copy(w16_sb[:], w16_ps[:])
                    wrep_ps = wrap_psum.tile([P, 8 * TOPK], f32, tag="wrep")
                    nc.tensor.matmul(wrep_ps[:], lhsT=rep[:], rhs=w16_sb[:],
                                     start=True, stop=True)
                    widx = small_pool.tile([P, 8 * TOPK], mybir.dt.int16,
                                           tag="widx")
                    nc.scalar.copy(widx[:], wrep_ps[:])  # f32 -> int16

                    # two 640-index gathers per tile on rotating SWDGE queues:
                    # W columns are r-major (m = r*8+u), so the first 40
                    # columns are ranks 0-4 and the last 40 are ranks 5-9.
                    g = gath_pool.tile([P, TOPK, D], bf16, tag="g")
                    half = TOPK // 2
                    for h in range(2):
                        nc.gpsimd.dma_gather(
                            out_ap=g[:, h * half:(h + 1) * half, :],
                            in_ap=xheadb[:],
                            idxs_ap=widx[:, h * 40:(h + 1) * 40],
                            num_idxs=half * P,
                            num_idxs_reg=half * P,
                            elem_size=D,
                            single_packet=False,
                            queue_num=(2 * i + h) % 4,
                        )

                    gf = gf_pool.tile([P, TOPK * D], f32, tag="gf")
                    nc.scalar.copy(gf[:], g[:].rearrange("p r d -> p (r d)"))

                    out_view = out[i * P * TOPK:(i + 1) * P * TOPK, :].rearrange(
                        "(p r) d -> p (r d)", p=P)
                    nc.sync.dma_start(out_view, gf[:])

    nc.compile()
    _BUILD_CACHE[n_points] = nc
    return nc


def _to_bf16(a: np.ndarray) -> np.ndarray:
    """fp32 -> bf16 with round-to-nearest-even, as a bfloat16 ndarray."""
    import ml_dtypes

    return a.astype(ml_dtypes.bfloat16)


def run_on_cores(x_np, cc_np, trace=False):
    """Run the SPMD kernel on all 8 cores. Returns (out [N*10,D], results)."""
    from concourse import bass_utils

    nc = build_nc(N_SHARD)
    xheadb = np.ascontiguousarray(_to_bf16(x_np[:K]))
    in_maps = [
        {
            "x": np.ascontiguousarray(x_np[c * N_SHARD:(c + 1) * N_SHARD]),
            "xheadb": xheadb,
            "cluster_centers": cc_np,
        }
        for c in range(N_CORES)
    ]
    res = bass_utils.run_bass_kernel_spmd(
        nc, in_maps, core_ids=list(range(N_CORES)), trace=trace,
    )
    shards = [res.results[c]["out"] for c in range(N_CORES)]
    full = np.concatenate(shards, axis=0)  # [N*10, D]
    return full, res


def kernel(x, cluster_centers):
    x_np = np.ascontiguousarray(np.asarray(x, dtype=np.float32))
    cc_np = np.ascontiguousarray(np.asarray(cluster_centers, dtype=np.float32))
    full, _ = run_on_cores(x_np, cc_np, trace=False)
    return full.reshape(1, N_FULL * TOPK, D)


# revision 12
# speedup vs baseline: 2.0009x; 1.8655x over previous
"""Trainium2 Bass kernel for DifferentiableKMeans retrieval-knn.

Computes, for x [32768, 128] and cluster_centers [512, 128]:
    d2 = ||x||^2 - 2 x@c.T + ||c||^2          [N, 512]
    idx = top-10 smallest-distance cluster ids  [N, 10]
    out = x[idx].reshape(1, N*10, 128)          (gather of x rows 0..511)

Sharding: data-parallel over N across 8 NeuronCores; centers and the
gather table (x[:512], pre-rounded to bf16 on host) replicated.

Per-core pipeline (4096 points, 32 tiles of 128):
  PE:  fp32 matmul m = x@cT, + K=1 matmul adds -0.5*|c|^2 (fp32r was
       tried and is a reduced-precision mode: rel err 0.11, rejected).
  DVE: max8 / max_index / match_replace / max8 / max_index => top-10
       indices (reads scores straight from PSUM).
  ACT: small copies, bf16 -> fp32 upcast of the gathered rows.
  GpSimd: 2 x 640-index dma_gather per tile (bf16 rows, 256B each),
       rotated over 4 SWDGE queues with deep buffering -- measured ~3x
       faster than one 1280-index gather on a single queue.

Ranking is by m = x.c - 0.5*|c|^2 (monotone equivalent of distance per
row). Gathered values are bf16-rounded (rel err ~1e-3, well under the
2e-2 gate); the output tensor itself is fp32.
"""

import os
import sys

for _p in ("/opt/trn_rl_repo", "/root/.axon_site/_ro/trn_rl_repo"):
    if os.path.isdir(_p) and _p not in sys.path:
        sys.path.insert(0, _p)

import numpy as np

N_FULL = 32768
D = 128
K = 512
TOPK = 10
N_CORES = 8
N_SHARD = N_FULL // N_CORES  # 4096
P = 128

_BUILD_CACHE = {}


def build_nc(n_points=N_SHARD):
    """Build (and compile) the single-core Bass module for a shard of
    `n_points` points. Returns the compiled Bacc instance."""
    if n_points in _BUILD_CACHE:
        return _BUILD_CACHE[n_points]

    import concourse.bass as bass
    import concourse.mybir as mybir
    from concourse import bacc
    from concourse.masks import make_identity
    from concourse.tile import TileContext

    f32 = mybir.dt.float32
    f32r = mybir.dt.float32r
    f16 = mybir.dt.float16
    bf16 = mybir.dt.bfloat16
    AFT = mybir.ActivationFunctionType
    nt = n_points // P
    assert n_points % P == 0

    nc = bacc.Bacc("TRN2", target_bir_lowering=False, debug=False,
                   num_swdge_queues=4)

    x = nc.dram_tensor("x", [n_points, D], f32, kind="ExternalInput")
    xheadb = nc.dram_tensor("xheadb", [K, D], bf16, kind="ExternalInput")
    cc = nc.dram_tensor("cluster_centers", [K, D], f32, kind="ExternalInput")
    out = nc.dram_tensor("out", [n_points * TOPK, D], bf16, kind="ExternalOutput")

    with TileContext(nc) as tc:
        with tc.tile_pool(name="const", bufs=1) as const_pool:
            identity = const_pool.tile([P, P], f32)
            make_identity(nc, identity[:])

            cT = const_pool.tile([P, K], f32)          # centers transposed [d, k]
            negc2 = const_pool.tile([1, K], f32)       # -0.5*|c_k|^2 row
            negc2bc = const_pool.tile([P, K], f32)     # broadcast over partitions
            ones_row = const_pool.tile([1, P], f32)
            nc.vector.memset(ones_row[:], 1.0)
            c2col = const_pool.tile([P, K // P], f32)

            # REP[k, m] = 1 iff m % 16 == k: replicates a [16, S] tile to all
            # 8 Q7-core partition blocks via matmul (out[m] = in[m % 16]).
            rep = const_pool.tile([16, P], f32)
            rep3 = rep[:].rearrange("k (u l) -> k u l", l=16)
            nc.gpsimd.memset(rep3, 0.0)
            # iota val = k - l; keep 0.0 where val != 0, write 1.0 where k == l
            nc.gpsimd.affine_select(
                out=rep3, in_=rep3,
                compare_op=mybir.AluOpType.not_equal, fill=1.0,
                base=0, pattern=[[0, P // 16], [-1, 16]], channel_multiplier=1,
            )

            with tc.tile_pool(name="prep", bufs=2) as prep_pool, \
                 tc.tile_pool(name="prep_ps", bufs=2, space="PSUM") as prep_psum:
                c2row_ps = prep_psum.tile([1, K], f32, tag="c2row")
                for j in range(K // P):
                    cchunk = prep_pool.tile([P, P], f32, tag="cchunk")
                    nc.sync.dma_start(cchunk[:], cc[j * P:(j + 1) * P, :])
                    pst = prep_psum.tile([P, P], f32, tag="pst")
                    nc.tensor.transpose(pst[:], cchunk[:], identity[:])
                    nc.scalar.copy(cT[:, j * P:(j + 1) * P], pst[:])
                    sq = prep_pool.tile([P, P], f32, tag="sq")
                    nc.scalar.activation(
                        sq[:], cchunk[:], AFT.Square,
                        accum_out=c2col[:, j:j + 1],
                    )
                    # transpose the [P,1] column of |c|^2 into a [1,P] row chunk
                    nc.tensor.matmul(
                        c2row_ps[0:1, j * P:(j + 1) * P],
                        lhsT=c2col[:, j:j + 1], rhs=identity[:],
                        start=True, stop=True,
                    )
                nc.scalar.activation(negc2[:], c2row_ps[:], AFT.Copy, scale=-0.5)
                bc_ps = prep_psum.tile([P, K], f32, tag="bc")
                nc.tensor.matmul(bc_ps[:], lhsT=ones_row[:], rhs=negc2[:],
                                 start=True, stop=True)
                nc.scalar.copy(negc2bc[:], bc_ps[:])

            with tc.tile_pool(name="xin", bufs=3) as xin_pool, \
                 tc.tile_pool(name="xt", bufs=3) as xt_pool, \
                 tc.tile_pool(name="ms2", bufs=3) as ms2_pool, \
                 tc.tile_pool(name="small", bufs=4) as small_pool, \
                 tc.tile_pool(name="gath", bufs=8) as gath_pool, \
                 tc.tile_pool(name="mm_ps", bufs=3, space="PSUM") as mm_psum, \
                 tc.tile_pool(name="wrap_ps", bufs=1, space="PSUM") as wrap_psum, \
                 tc.tile_pool(name="tr_ps", bufs=2, space="PSUM") as tr_psum:
                for i in range(nt):
                    x_tile = xin_pool.tile([P, D], f32, tag="x")
                    nc.sync.dma_start(x_tile[:], x[i * P:(i + 1) * P, :])

                    pst = tr_psum.tile([P, P], f32, tag="xtp")
                    nc.tensor.transpose(pst[:], x_tile[:], identity[:])
                    xT = xt_pool.tile([P, P], f32, tag="xT")
                    nc.scalar.copy(xT[:], pst[:])

                    pm = mm_psum.tile([P, K], f32, tag="pm")
                    nc.tensor.matmul(pm[:], lhsT=xT[:], rhs=cT[:],
                                     start=True, stop=True)

                    # bias-add on DVE (PSUM -> SBUF), then top-10 per point
                    ms = ms2_pool.tile([P, K], f32, tag="ms")
                    nc.vector.tensor_add(ms[:], pm[:], negc2bc[:])
                    v8 = small_pool.tile([P, 8], f32, tag="v8")
                    nc.vector.max(v8[:], ms[:])
                    idx = small_pool.tile([P, 16], mybir.dt.uint32, tag="idx")
                    nc.vector.max_index(idx[:, 0:8], v8[:], ms[:])
                    ms2 = ms2_pool.tile([P, K], f32, tag="ms2")
                    nc.vector.match_replace(
                        out=ms2[:], in_to_replace=v8[:], in_values=ms[:],
                        imm_value=-1e30,
                    )
                    v8b = small_pool.tile([P, 8], f32, tag="v8b")
                    nc.vector.max(v8b[:], ms2[:])
                    nc.vector.max_index(idx[:, 8:16], v8b[:], ms2[:])

                    # ---- build the 16-partition-wrapped int16 index list ----
                    # dma_gather consumes L[j] = W[j % 16, j // 16] and lands
                    # row j at out[j % 128, j // 128].  With j = r*128 + p the
                    # landing spot is (p, r), so we need
                    #   W[l, r*8 + u] = idx[l + 16*u, r]      (p = l + 16u)
                    # replicated across the 8 Q7-core partition blocks.
                    # fp16 keeps index values (< 512) exact and transposes at
                    # half the fp32 PE cost.
                    idxf = small_pool.tile([P, TOPK], f32, tag="idxf")
                    nc.scalar.copy(idxf[:], idx[:, 0:TOPK])  # uint32 -> f16
                    t_ps = wrap_psum.tile([TOPK, P], f32, tag="t_ps")
                    nc.tensor.transpose(t_ps[:], idxf[:], identity[:])
                    t_sb = small_pool.tile([TOPK, P], f32, tag="t_sb")
                    nc.scalar.copy(t_sb[:], t_ps[:])
                    w16_ps = wrap_psum.tile([16, 8 * TOPK], f32, tag="w16")
                    for u in range(8):
                        # [10, 16] slice -> transposed into columns r*8 + u
                        nc.tensor.transpose(
                            w16_ps[:, u:8 * TOPK:8],
                            t_sb[:, 16 * u:16 * (u + 1)],
                            identity[0:TOPK, 0:TOPK],
                        )
                    w16_sb = small_pool.tile([16, 8 * TOPK], f32, tag="w16sb")
                    nc.scalar.copy(w16_sb[:], w16_ps[:])
                    wrep_ps = wrap_psum.tile([P, 8 * TOPK], f32, tag="wrep")
                    nc.tensor.matmul(wrep_ps[:], lhsT=rep[:], rhs=w16_sb[:],
                                     start=True, stop=True)
                    widx = small_pool.tile([P, 8 * TOPK], mybir.dt.int16,
                                           tag="widx")
                    nc.scalar.copy(widx[:], wrep_ps[:])  # f32 -> int16

                    # two 640-index gathers per tile on rotating SWDGE queues:
                    # W columns are r-major (m = r*8+u), so the first 40
                    # columns are ranks 0-4 and the last 40 are ranks 5-9.
                    g = gath_pool.tile([P, TOPK, D], bf16, tag="g")
                    half = TOPK // 2
                    for h in range(2):
                        nc.gpsimd.dma_gather(
                            out_ap=g[:, h * half:(h + 1) * half, :],
                            in_ap=xheadb[:],
                            idxs_ap=widx[:, h * 40:(h + 1) * 40],
                            num_idxs=half * P,
                            num_idxs_reg=half * P,
                            elem_size=D,
                            single_packet=False,
                            queue_num=(2 * i + h) % 4,
                        )

                    out_view = out[i * P * TOPK:(i + 1) * P * TOPK, :].rearrange(
                        "(p r) d -> p (r d)", p=P)
                    nc.sync.dma_start(out_view, g[:].rearrange("p r d -> p (r d)"))

    nc.compile()
    _BUILD_CACHE[n_points] = nc
    return nc


def _to_bf16(a: np.ndarray) -> np.ndarray:
    """fp32 -> bf16 with round-to-nearest-even, as a bfloat16 ndarray."""
    import ml_dtypes

    return a.astype(ml_dtypes.bfloat16)


def run_on_cores(x_np, cc_np, trace=False):
    """Run the SPMD kernel on all 8 cores. Returns (out [N*10,D], results)."""
    from concourse import bass_utils

    nc = build_nc(N_SHARD)
    xheadb = np.ascontiguousarray(_to_bf16(x_np[:K]))
    in_maps = [
        {
            "x": np.ascontiguousarray(x_np[c * N_SHARD:(c + 1) * N_SHARD]),
            "xheadb": xheadb,
            "cluster_centers": cc_np,
        }
        for c in range(N_CORES)
    ]
    res = bass_utils.run_bass_kernel_spmd(
        nc, in_maps, core_ids=list(range(N_CORES)), trace=trace,
    )
    shards = [np.asarray(res.results[c]["out"], dtype=np.float32)
              for c in range(N_CORES)]
    full = np.concatenate(shards, axis=0)  # [N*10, D]
    return full, res


def kernel(x, cluster_centers):
    x_np = np.ascontiguousarray(np.asarray(x, dtype=np.float32))
    cc_np = np.ascontiguousarray(np.asarray(cluster_centers, dtype=np.float32))
    full, _ = run_on_cores(x_np, cc_np, trace=False)
    return full.reshape(1, N_FULL * TOPK, D)
